# revision 1
# baseline (speedup 1.0000x reference)
"""Trainium2 Bass kernel for nn_DistillationSingleClassDetectionLoss.

Strategy: data-parallel over batch N=32 -> 8 cores x 4 images. Each core runs
the full per-image SimOTA assignment + loss for its 4 images and outputs 4
per-image losses plus partial sums; host averages the 32 losses.

Layouts:
  L1: [128 partitions = a-chunk, 66 tiles, 48 gts]  (A padded 8400 -> 8448)
  L2 (transposed, for per-gt top-k): [128 partitions, 4224]
      rows 0:48 = gt m for a-tiles 0:33, rows 64:112 = gt m for tiles 33:66
      (rows 48:64 / 112:128 hold initialized junk from 64-wide transposes)
"""
import numpy as np
from contextlib import ExitStack

import concourse.bass as bass
import concourse.bacc as bacc
import concourse.tile as tile
import concourse.mybir as mybir

F32 = mybir.dt.float32
ALU = mybir.AluOpType
ACTF = mybir.ActivationFunctionType
AXL = mybir.AxisListType

NIMG = 4          # images per core
A = 8400
AP_ = 8448        # padded
T = 66            # a-tiles
M = 48            # gts
HALF = 33         # tiles per half
NG = 9            # g-quantities: gx1 gy1 gx2 gy2 gcx gcy wg2 hg2 SG'
GRP = 11          # tiles per transpose group (3 groups per half)
FLAT = T * M      # 3168
FLATP = FLAT + 16  # slack so 64-wide transpose views stay in-bounds

_CACHED = {}


def build_nc(repeat=1):
    nc = bacc.Bacc("TRN2", target_bir_lowering=False, debug=False)
    V, G, S, PE, DMA = nc.vector, nc.gpsimd, nc.scalar, nc.tensor, nc.sync

    # ---------------- DRAM I/O ----------------
    d_sd = nc.dram_tensor("sd", [NIMG, 5, 128, T], F32, kind="ExternalInput")
    d_td = nc.dram_tensor("td", [NIMG, 5, 128, T], F32, kind="ExternalInput")
    d_sp = nc.dram_tensor("sp", [NIMG, 128, T], F32, kind="ExternalInput")
    d_tp = nc.dram_tensor("tp", [NIMG, 128, T], F32, kind="ExternalInput")
    d_tg = nc.dram_tensor("tg", [NIMG, M, 4], F32, kind="ExternalInput")
    d_pri = nc.dram_tensor("pri", [4, 128, T], F32, kind="ExternalInput")
    d_eye = nc.dram_tensor("eye", [128, 128], F32, kind="ExternalInput")
    d_iotam = nc.dram_tensor("iotam", [128, M], F32, kind="ExternalInput")
    d_iotap = nc.dram_tensor("iotap", [128, 1], F32, kind="ExternalInput")
    d_iota8 = nc.dram_tensor("iota8", [128, 8], F32, kind="ExternalInput")

    d_gscr = nc.dram_tensor("gscr", [NIMG, NG * M], F32, kind="Internal")
    d_tau = nc.dram_tensor("tauscr", [NIMG, 2, M], F32, kind="Internal")
    d_gf = nc.dram_tensor("gfscr", [NIMG, AP_], F32, kind="Internal")

    d_out = nc.dram_tensor("out_losses", [1, NIMG], F32, kind="ExternalOutput")
    d_res = nc.dram_tensor("out_res", [1, 24], F32, kind="ExternalOutput")

    with tile.TileContext(nc) as tc, ExitStack() as ctx:
        persist = ctx.enter_context(tc.tile_pool(name="persist", bufs=1))
        inp = ctx.enter_context(tc.tile_pool(name="inp", bufs=2))
        mat = ctx.enter_context(tc.tile_pool(name="mat", bufs=1))
        sm = ctx.enter_context(tc.tile_pool(name="sm", bufs=2))
        psum = ctx.enter_context(tc.tile_pool(name="ps", bufs=2, space="PSUM"))
        psum_s = ctx.enter_context(tc.tile_pool(name="pss", bufs=2, space="PSUM"))

        # ---------------- constants ----------------
        eye = persist.tile([128, 128], F32, tag="eye", name="eye")
        DMA.dma_start(eye[:], d_eye.ap())
        iotam = persist.tile([128, M], F32, tag="iotam", name="iotam")
        DMA.dma_start(iotam[:], d_iotam.ap())
        iotap = persist.tile([128, 1], F32, tag="iotap", name="iotap")
        DMA.dma_start(iotap[:], d_iotap.ap())
        iota8 = persist.tile([128, 8], F32, tag="iota8", name="iota8")
        DMA.dma_start(iota8[:], d_iota8.ap())
        zero1 = persist.tile([128, 1], F32, tag="zero1", name="zero1")
        V.memset(zero1[:], 0.0)
        eps1 = persist.tile([128, 1], F32, tag="eps1", name="eps1")
        V.memset(eps1[:], 1e-7)
        ones1 = persist.tile([128, 1], F32, tag="ones1", name="ones1")
        V.memset(ones1[:], 1.0)
        res = persist.tile([128, 24], F32, tag="res", name="res")

        # prior-derived vectors (device compute, tiny)
        p0 = sm.tile([128, T], F32, tag="p0", name="p0")
        DMA.dma_start(p0[:], d_pri.ap()[0])
        p1 = sm.tile([128, T], F32, tag="p1", name="p1")
        DMA.dma_start(p1[:], d_pri.ap()[1])
        p2 = sm.tile([128, T], F32, tag="p2", name="p2")
        DMA.dma_start(p2[:], d_pri.ap()[2])
        cxv = persist.tile([128, T], F32, tag="cxv", name="cxv")
        V.scalar_tensor_tensor(cxv[:], p2[:], 0.5, p0[:], ALU.mult, ALU.add)
        cyv = persist.tile([128, T], F32, tag="cyv", name="cyv")
        V.scalar_tensor_tensor(cyv[:], p2[:], 0.5, p1[:], ALU.mult, ALU.add)
        Rv = persist.tile([128, T], F32, tag="Rv", name="Rv")
        V.tensor_scalar(Rv[:], p2[:], 2.5, None, ALU.mult)

        def ab(x):  # a-vec [128, T] -> bcast [128, T, M]
            return x.unsqueeze(2).broadcast_to([128, T, M])

        def m3(t):  # flat big tile -> [128, T, M] view
            return t[:, 0:FLAT].rearrange("p (t m) -> p t m", m=M)

        cxb = ab(cxv[:])
        cyb = ab(cyv[:])
        Rb = ab(Rv[:])

        def new_mat(tag):
            return mat.tile([128, FLAT], F32, tag=tag, name=tag)

        # =========================================================
        # per-image pipeline
        # =========================================================
        for i in [im for _ in range(repeat) for im in range(NIMG)]:
            # ---- load inputs ----
            sdp = [inp.tile([128, T], F32, tag=f"sdp{j}", name=f"sdp{j}") for j in range(5)]
            for j in range(5):
                DMA.dma_start(sdp[j][:], d_sd.ap()[i, j])
            tdp = [inp.tile([128, T], F32, tag=f"tdp{j}", name=f"tdp{j}") for j in range(5)]
            for j in range(5):
                DMA.dma_start(tdp[j][:], d_td.ap()[i, j])
            spt = inp.tile([128, T], F32, tag="spt", name="spt")
            DMA.dma_start(spt[:], d_sp.ap()[i])
            tpt = inp.tile([128, T], F32, tag="tpt", name="tpt")
            DMA.dma_start(tpt[:], d_tp.ap()[i])

            # ---- G prep (partition-0 row math) ----
            tgrow = sm.tile([1, M * 4], F32, tag="tgrow", name="tgrow")
            DMA.dma_start(tgrow[:], d_tg.ap()[i].rearrange("m c -> (m c)").unsqueeze(0))
            grow = sm.tile([1, NG, M], F32, tag="grow", name="grow")
            # coords: grow[0, 0:4, m] = tg[m, c]
            V.tensor_copy(grow[0:1, 0:4, :],
                          tgrow[0:1, :].rearrange("p (m c) -> p c m", c=4))
            # centers: (x1+x2)*0.5, (y1+y2)*0.5
            V.tensor_tensor(grow[0:1, 4:6, :], grow[0:1, 0:2, :],
                            grow[0:1, 2:4, :], ALU.add)
            # full w/h first (for SG), then halve
            V.tensor_tensor(grow[0:1, 6:8, :], grow[0:1, 2:4, :],
                            grow[0:1, 0:2, :], ALU.subtract)
            V.tensor_tensor(grow[0:1, 8:9, :], grow[0:1, 6:7, :],
                            grow[0:1, 7:8, :], ALU.mult)
            V.tensor_scalar(grow[0:1, 8:9, :], grow[0:1, 8:9, :], 1e-6, None, ALU.add)
            V.tensor_scalar(grow[0:1, 4:8, :], grow[0:1, 4:8, :], 0.5, None, ALU.mult)
            DMA.dma_start(d_gscr.ap()[i].unsqueeze(0), grow[0:1, :, :].rearrange("p a b -> p (a b)"))
            Gg = sm.tile([128, NG, M], F32, tag="Gg", name="Gg")
            DMA.dma_start(Gg[:], d_gscr.ap()[i].rearrange("(a b) -> a b", b=M)
                          .unsqueeze(0).broadcast_to([128, NG, M]))

            def gbb(q):  # g-vec bcast [128, T, M]
                return Gg[:, q:q + 1, :].broadcast_to([128, T, M])

            # tgts rhs tiles for PE gather (both partition bases), zero junk rows
            tgts4 = sm.tile([128, 4], F32, tag="tgts4", name="tgts4")
            V.memset(tgts4[:], 0.0)
            DMA.dma_start(tgts4[0:48, :], d_tg.ap()[i])
            DMA.dma_start(tgts4[64:112, :], d_tg.ap()[i])

            # ---- masks ----
            Dx = new_mat("tmpA")
            V.tensor_tensor(m3(Dx), cxb, gbb(4), ALU.subtract)
            Dy = new_mat("tmpB")
            V.tensor_tensor(m3(Dy), cyb, gbb(5), ALU.subtract)
            AXm = new_mat("tmpC")
            S.activation(m3(AXm), m3(Dx), ACTF.Abs, bias=zero1[:])
            AYm = new_mat("tmpD")
            S.activation(m3(AYm), m3(Dy), ACTF.Abs, bias=zero1[:])
            mxy = new_mat("tmpA")
            V.tensor_tensor(m3(mxy), m3(AXm), m3(AYm), ALU.max)
            ggx = new_mat("tmpB")
            V.scalar_tensor_tensor(m3(ggx), m3(AXm), -1.0, gbb(6), ALU.mult, ALU.add)
            ggy = new_mat("tmpC")
            V.scalar_tensor_tensor(m3(ggy), m3(AYm), -1.0, gbb(7), ALU.mult, ALU.add)
            g_gt = new_mat("tmpD")
            V.tensor_tensor(m3(g_gt), m3(ggx), m3(ggy), ALU.min)
            g_ct = new_mat("tmpB")
            V.scalar_tensor_tensor(m3(g_ct), m3(mxy), -1.0, Rb, ALU.mult, ALU.add)
            ib = new_mat("tmpA")
            V.tensor_tensor(m3(ib), m3(g_gt), m3(g_ct), ALU.min)
            vg = new_mat("tmpC")
            V.tensor_tensor(m3(vg), m3(g_gt), m3(g_ct), ALU.max)
            pen = mat.tile([128, FLAT], F32, tag="pen", name="pen")
            V.tensor_scalar(m3(pen), m3(ib), 0.0, 1e5, ALU.is_le, ALU.mult)
            vmax = sm.tile([128, T], F32, tag="vmax", name="vmax")
            V.tensor_reduce(vmax[:], m3(vg), AXL.X, ALU.max)
            valid = sm.tile([128, T], F32, tag="valid", name="valid")
            V.tensor_scalar(valid[:], vmax[:], 0.0, None, ALU.is_gt)
            vpen = sm.tile([128, T], F32, tag="vpen", name="vpen")
            V.tensor_scalar(vpen[:], valid[:], -1e8, 1e8, ALU.mult, ALU.add)

            # per-image loss accumulators (written by assignment closure)
            img_out = {}

            # ---- one assignment ----
            def assignment(planes, aidx, resolve):
                score, ax1, ay1, ax2, ay2 = (planes[j][:] for j in range(5))
                # per-a smalls
                sc = sm.tile([128, T], F32, tag="sc", name="sc")
                V.tensor_scalar(sc[:], score, 1e-12, None, ALU.max)
                lsc = sm.tile([128, T], F32, tag="lsc", name="lsc")
                S.activation(lsc[:], sc[:], ACTF.Ln, bias=zero1[:])
                clsv = sm.tile([128, T], F32, tag="clsv", name="clsv")
                V.scalar_tensor_tensor(clsv[:], lsc[:], -0.5, vpen[:], ALU.mult, ALU.add)
                aw = sm.tile([128, T], F32, tag="aw", name="aw")
                V.tensor_tensor(aw[:], ax2, ax1, ALU.subtract)
                ah = sm.tile([128, T], F32, tag="ah", name="ah")
                V.tensor_tensor(ah[:], ay2, ay1, ALU.subtract)
                SA = sm.tile([128, T], F32, tag="SA", name="SA")
                V.tensor_tensor(SA[:], aw[:], ah[:], ALU.mult)

                # ---- iou matrix ----
                t1 = new_mat("tmpA")
                V.tensor_tensor(m3(t1), ab(ax2), gbb(2), ALU.min)
                t2 = new_mat("tmpB")
                V.tensor_tensor(m3(t2), ab(ax1), gbb(0), ALU.max)
                wr = new_mat("tmpC")
                V.tensor_tensor(m3(wr), m3(t1), m3(t2), ALU.subtract)
                t3 = new_mat("tmpA")
                V.tensor_tensor(m3(t3), ab(ay2), gbb(3), ALU.min)
                t4 = new_mat("tmpB")
                V.tensor_tensor(m3(t4), ab(ay1), gbb(1), ALU.max)
                hr = new_mat("tmpD")
                V.tensor_tensor(m3(hr), m3(t3), m3(t4), ALU.subtract)
                w_ = new_mat("tmpA")
                S.activation(m3(w_), m3(wr), ACTF.Relu, bias=zero1[:])
                h_ = new_mat("tmpB")
                S.activation(m3(h_), m3(hr), ACTF.Relu, bias=zero1[:])
                inter = new_mat("tmpC")
                V.tensor_tensor(m3(inter), m3(w_), m3(h_), ALU.mult)
                # fold the valid mask into the denominator: SA2 = SA + (1-valid)*1e8
                # makes iou <= ~2e-4 for invalid priors (vs 0 in the reference);
                # they still can't match (cost penalty dominates), and the
                # perturbation to per-gt top-10 iou sums is << the int-trunc step.
                SA2 = sm.tile([128, T], F32, tag="SA2", name="SA2")
                V.tensor_tensor(SA2[:], SA[:], vpen[:], ALU.add)
                sgmi = new_mat("tmpA")
                V.scalar_tensor_tensor(m3(sgmi), m3(inter), -1.0, gbb(8), ALU.mult, ALU.add)
                union = new_mat("tmpB")
                V.tensor_tensor(m3(union), m3(sgmi), ab(SA2[:]), ALU.add)
                rcp = new_mat("tmpA")
                V.reciprocal(m3(rcp), m3(union))
                ioum = mat.tile([128, FLATP], F32, tag="ioum", name="ioum")
                V.memset(ioum[:, FLAT:FLATP], -1e30)
                V.tensor_tensor(m3(ioum), m3(inter), m3(rcp), ALU.mult)

                # ---- cost ----
                L = new_mat("tmpA")
                S.activation(m3(L), m3(ioum), ACTF.Ln, bias=eps1[:])
                nc1 = new_mat("tmpB")
                V.scalar_tensor_tensor(m3(nc1), m3(L), 3.0, m3(pen), ALU.mult, ALU.subtract)
                negc = mat.tile([128, FLATP], F32, tag="negc", name="negc")
                V.memset(negc[:, FLAT:FLATP], -1e30)
                V.tensor_tensor(m3(negc), m3(nc1), ab(clsv[:]), ALU.subtract)

                # ---- transposes + topk chains ----
                ioT = mat.tile([128, HALF * 128], F32, tag="ioT", name="ioT")
                UN = sm.tile([128, 48], F32, tag="UN", name="UN")
                for h in range(2):
                    for g in range(3):
                        cols = slice(1408 * g, 1408 * (g + 1))
                        if h == 0:
                            psg = psum.tile([64, GRP * 128], F32, tag="psg", name="psg")
                            for j in range(GRP):
                                t = GRP * g + j
                                PE.transpose(psg[0:64, 128 * j:128 * (j + 1)],
                                             ioum[:, M * t:M * t + 64], eye[:])
                            S.activation(ioT[0:64, cols], psg[0:64, :], ACTF.Copy)
                        else:
                            # transpose a 128-col window starting 64 cols early:
                            # output partitions 64:128 carry tile t's gt rows, so
                            # the evict lands directly in ioT rows 64+ (no shift DMA)
                            psg = psum.tile([128, GRP * 128], F32, tag="psg", name="psg")
                            for j in range(GRP):
                                t = HALF + GRP * g + j
                                PE.transpose(psg[:, 128 * j:128 * (j + 1)],
                                             ioum[:, M * t - 64:M * t + 64], eye[:])
                            S.activation(ioT[64:128, cols], psg[64:128, :], ACTF.Copy)
                        # negcost group -> psum -> group max (top-8)
                        psn = psum.tile([64, GRP * 128], F32, tag="psg", name="psn")
                        for j in range(GRP):
                            t = HALF * h + GRP * g + j
                            PE.transpose(psn[0:64, 128 * j:128 * (j + 1)],
                                         negc[:, M * t:M * t + 64], eye[:])
                        V.max(UN[0:64, 8 * (3 * h + g):8 * (3 * h + g) + 8],
                              psn[0:64, :])

                # iou chain: top-16 per half
                UI = sm.tile([128, 32], F32, tag="UI", name="UI")
                V.max(UI[:, 0:8], ioT[:])
                iorep = mat.tile([128, HALF * 128], F32, tag="scr2", name="scr2")
                V.match_replace(iorep[:], UI[:, 0:8], ioT[:], -1e30)
                V.max(UI[:, 8:16], iorep[:])
                DMA.dma_start(UI[0:48, 16:32], UI[64:112, 0:16])
                F8 = sm.tile([128, 8], F32, tag="F8", name="F8")
                V.max(F8[0:48, :], UI[0:48, :])
                UIrep = sm.tile([128, 32], F32, tag="UIrep", name="UIrep")
                V.match_replace(UIrep[0:48, :], F8[0:48, :], UI[0:48, :], -1e30)
                F8b = sm.tile([128, 8], F32, tag="F8b", name="F8b")
                V.max(F8b[0:48, :], UIrep[0:48, :])
                S10 = sm.tile([128, 1], F32, tag="S10", name="S10")
                V.tensor_reduce(S10[0:48, :], F8[0:48, :], AXL.X, ALU.add)
                S10b = sm.tile([128, 1], F32, tag="S10b", name="S10b")
                V.tensor_reduce(S10b[0:48, :], F8b[0:48, 0:2], AXL.X, ALU.add)
                V.tensor_tensor(S10[0:48, :], S10[0:48, :], S10b[0:48, :], ALU.add)

                # negcost merge + tau select
                NC8 = sm.tile([128, 8], F32, tag="NC8", name="NC8")
                V.max(NC8[0:48, :], UN[0:48, :])
                jsel = sm.tile([128, 1], F32, tag="jsel", name="jsel")
                V.tensor_scalar(jsel[0:48, :], S10[0:48, :], 1.0, 1.0, ALU.max, ALU.subtract)
                tsel = sm.tile([128, 8], F32, tag="tsel", name="tsel")
                V.tensor_scalar(tsel[0:48, :], iota8[0:48, :], -1.0, jsel[0:48, :], ALU.mult, ALU.add)
                oh1 = sm.tile([128, 8], F32, tag="oh1", name="oh1")
                V.tensor_scalar(oh1[0:48, :], tsel[0:48, :], 0.0, None, ALU.is_ge)
                oh2 = sm.tile([128, 8], F32, tag="oh2", name="oh2")
                V.tensor_scalar(oh2[0:48, :], tsel[0:48, :], 1.0, None, ALU.is_lt)
                V.tensor_tensor(oh1[0:48, :], oh1[0:48, :], oh2[0:48, :], ALU.mult)
                V.tensor_tensor(oh1[0:48, :], oh1[0:48, :], NC8[0:48, :], ALU.mult)
                tau = sm.tile([128, 1], F32, tag="tau", name="tau")
                V.tensor_reduce(tau[0:48, :], oh1[0:48, :], AXL.X, ALU.add)
                DMA.dma_start(d_tau.ap()[i, aidx], tau[0:48, :])
                Tgb = sm.tile([128, M], F32, tag="Tgb", name="Tgb")
                DMA.dma_start(Tgb[:], d_tau.ap()[i, aidx].unsqueeze(0)
                              .broadcast_to([128, M]))

                # ---- matching + count ----
                matching = new_mat("tmpA")
                V.scalar_tensor_tensor(m3(matching), m3(negc), 0.0, Tgb[:].unsqueeze(1).broadcast_to([128, T, M]),
                                       ALU.add, ALU.is_ge)
                count = sm.tile([128, T], F32, tag="count", name="count")
                V.tensor_reduce(count[:], m3(matching), AXL.X, ALU.add)
                fg = sm.tile([128, T], F32, tag=f"fg{aidx}")
                V.tensor_scalar(fg[:], count[:], 1.0, None, ALU.is_ge)
                img_out[f"fg{aidx}"] = fg

                if not resolve:
                    return

                multi = sm.tile([128, T], F32, tag="multi", name="multi")
                V.tensor_scalar(multi[:], count[:], 1.0, None, ALU.is_gt)
                rowmax = sm.tile([128, T], F32, tag="rowmax", name="rowmax")
                V.tensor_reduce(rowmax[:], m3(negc), AXL.X, ALU.max)
                eq = new_mat("tmpB")
                V.tensor_tensor(m3(eq), m3(negc), ab(rowmax[:]), ALU.is_equal)
                mcol1 = new_mat("tmpC")
                V.scalar_tensor_tensor(m3(mcol1), m3(eq), -64.0,
                                       iotam[:].unsqueeze(1).broadcast_to([128, T, M]),
                                       ALU.mult, ALU.add)
                g1 = sm.tile([128, T], F32, tag="g1", name="g1")
                V.tensor_reduce(g1[:], m3(mcol1), AXL.X, ALU.min)
                mcol2 = new_mat("tmpB")
                V.scalar_tensor_tensor(m3(mcol2), m3(matching), -64.0,
                                       iotam[:].unsqueeze(1).broadcast_to([128, T, M]),
                                       ALU.mult, ALU.add)
                g2 = sm.tile([128, T], F32, tag="g2", name="g2")
                V.tensor_reduce(g2[:], m3(mcol2), AXL.X, ALU.min)
                gF = sm.tile([128, T], F32, tag="gF", name="gF")
                V.tensor_tensor(gF[:], g1[:], g2[:], ALU.subtract)
                V.tensor_tensor(gF[:], gF[:], multi[:], ALU.mult)
                V.tensor_tensor(gF[:], gF[:], g2[:], ALU.add)
                V.tensor_scalar(gF[:], gF[:], 64.0, None, ALU.add)

                # gF -> L2 broadcast via PE transpose + DRAM bounce
                psgf = psum_s.tile([T, 128], F32, tag="sps", name="sps")
                PE.transpose(psgf[:], gF[:], eye[:])
                gFT = sm.tile([T, 128], F32, tag="gFT", name="gFT")
                S.activation(gFT[:], psgf[:], ACTF.Copy)
                DMA.dma_start(d_gf.ap()[i].rearrange("(a b) -> a b", b=128), gFT[:])
                gFB = mat.tile([128, HALF * 128], F32, tag="scr2", name="scr2")
                for h in range(2):
                    src = (d_gf.ap()[i][h * HALF * 128:(h + 1) * HALF * 128]
                           .rearrange("(t i) -> t i", i=128)
                           .unsqueeze(0).broadcast_to([64, HALF, 128]))
                    DMA.dma_start(gFB[64 * h:64 * h + 64, :]
                                  .rearrange("p (t i) -> p t i", i=128), src)
                onehotT = mat.tile([128, HALF * 128], F32, tag="onehotT", name="onehotT")
                V.tensor_tensor(onehotT[:], iotap[:].broadcast_to([128, HALF * 128]),
                                gFB[:], ALU.is_equal)
                prodT = mat.tile([128, HALF * 128], F32, tag="scr2", name="scr2")
                V.tensor_tensor(prodT[:], onehotT[:], ioT[:], ALU.mult)

                # PE contractions: tgt boxes [128, T, 4] and matched_iou [128, T]
                pstgt = psum.tile([128, T * 4], F32, tag="psg", name="pstgt")
                psmi = psum_s.tile([128, T], F32, tag="sps", name="sps")
                for t in range(T):
                    h = t // HALF
                    pb2 = 64 * h
                    cols = slice(128 * (t % HALF), 128 * (t % HALF + 1))
                    PE.matmul(pstgt[:, 4 * t:4 * (t + 1)],
                              onehotT[pb2:pb2 + 64, cols], tgts4[pb2:pb2 + 64, :],
                              start=True, stop=True)
                    PE.matmul(psmi[:, t:t + 1],
                              prodT[pb2:pb2 + 64, cols], ones1[pb2:pb2 + 64, :],
                              start=True, stop=True)
                tgtb = sm.tile([128, T, 4], F32, tag="tgtb", name="tgtb")
                S.activation(tgtb[:].rearrange("p t c -> p (t c)"), pstgt[:], ACTF.Copy)
                ct = sm.tile([128, T], F32, tag="ct", name="ct")
                V.tensor_tensor(ct[:], psmi[:], fg[:], ALU.mult)
                img_out["conf_target"] = ct
                img_out["tgtb"] = tgtb

            assignment(sdp, 0, True)
            assignment(tdp, 1, False)

            # ================= losses =================
            def smt(tag):
                return sm.tile([128, T], F32, tag=tag, name=tag)

            # focal shared pieces on student logits x = spt
            x = spt[:]
            p_ = smt("p_")
            S.activation(p_[:], x, ACTF.Sigmoid, bias=zero1[:])
            relux = smt("relux")
            S.activation(relux[:], x, ACTF.Relu, bias=zero1[:])
            absx = smt("absx")
            S.activation(absx[:], x, ACTF.Abs, bias=zero1[:])
            expx = smt("expx")
            S.activation(expx[:], absx[:], ACTF.Exp, bias=zero1[:], scale=-1.0)
            spx = smt("spx")
            S.activation(spx[:], expx[:], ACTF.Ln, bias=ones1[:])
            sigt = smt("sigt")
            S.activation(sigt[:], tpt[:], ACTF.Sigmoid, bias=zero1[:])

            def focal(tgt, rescol):
                u = smt("fu")
                V.tensor_scalar(u[:], p_[:], -2.0, 1.0, ALU.mult, ALU.add)
                w1 = smt("fw1")
                V.tensor_tensor(w1[:], tgt[:], u[:], ALU.mult)
                omp = smt("fomp")
                V.tensor_tensor(omp[:], p_[:], w1[:], ALU.add)
                xt = smt("fxt")
                V.tensor_tensor(xt[:], x, tgt[:], ALU.mult)
                ce = smt("fce")
                V.scalar_tensor_tensor(ce[:], xt[:], -1.0, relux[:], ALU.mult, ALU.add)
                V.tensor_tensor(ce[:], ce[:], spx[:], ALU.add)
                at = smt("fat")
                V.tensor_scalar(at[:], tgt[:], -0.5, 0.75, ALU.mult, ALU.add)
                o2 = smt("fo2")
                V.tensor_tensor(o2[:], omp[:], omp[:], ALU.mult)
                V.tensor_tensor(o2[:], o2[:], ce[:], ALU.mult)
                V.tensor_tensor(o2[:], o2[:], at[:], ALU.mult)
                V.tensor_reduce(res[:, rescol:rescol + 1], o2[:], AXL.X, ALU.add)

            focal(img_out["conf_target"], 0 + i)   # F1
            focal(sigt, 4 + i)                      # F2

            # eiou shared (student pred boxes)
            px1, py1, px2, py2 = sdp[1][:], sdp[2][:], sdp[3][:], sdp[4][:]
            pw = smt("pw")
            V.tensor_tensor(pw[:], px2, px1, ALU.subtract)
            ph = smt("ph")
            V.tensor_tensor(ph[:], py2, py1, ALU.subtract)
            pa = smt("pa")
            V.tensor_tensor(pa[:], pw[:], ph[:], ALU.mult)
            psx = smt("psx")
            V.tensor_tensor(psx[:], px1, px2, ALU.add)
            psy = smt("psy")
            V.tensor_tensor(psy[:], py1, py2, ALU.add)

            def eiou(tx1, tx2, ty1, ty2, fg, ecol, ccol):
                e1 = smt("e1")
                V.tensor_tensor(e1[:], px2, tx2, ALU.min)
                e2 = smt("e2")
                V.tensor_tensor(e2[:], px1, tx1, ALU.max)
                iw = smt("iw")
                V.tensor_tensor(iw[:], e1[:], e2[:], ALU.subtract)
                V.tensor_scalar(iw[:], iw[:], 0.0, None, ALU.max)
                V.tensor_tensor(e1[:], py2, ty2, ALU.min)
                V.tensor_tensor(e2[:], py1, ty1, ALU.max)
                ih = smt("ih")
                V.tensor_tensor(ih[:], e1[:], e2[:], ALU.subtract)
                V.tensor_scalar(ih[:], ih[:], 0.0, None, ALU.max)
                inte = smt("inte")
                V.tensor_tensor(inte[:], iw[:], ih[:], ALU.mult)
                tw = smt("tw")
                V.tensor_tensor(tw[:], tx2, tx1, ALU.subtract)
                th = smt("th")
                V.tensor_tensor(th[:], ty2, ty1, ALU.subtract)
                ta = smt("ta")
                V.tensor_tensor(ta[:], tw[:], th[:], ALU.mult)
                un = smt("un")
                V.tensor_tensor(un[:], pa[:], ta[:], ALU.add)
                V.tensor_tensor(un[:], un[:], inte[:], ALU.subtract)
                V.tensor_scalar(un[:], un[:], 1e-7, None, ALU.add)
                r_ = smt("r_")
                V.reciprocal(r_[:], un[:])
                iouv = smt("iouv")
                V.tensor_tensor(iouv[:], inte[:], r_[:], ALU.mult)
                cw = smt("cw")
                V.tensor_tensor(e1[:], px2, tx2, ALU.max)
                V.tensor_tensor(e2[:], px1, tx1, ALU.min)
                V.tensor_tensor(cw[:], e1[:], e2[:], ALU.subtract)
                ch = smt("ch")
                V.tensor_tensor(e1[:], py2, ty2, ALU.max)
                V.tensor_tensor(e2[:], py1, ty1, ALU.min)
                V.tensor_tensor(ch[:], e1[:], e2[:], ALU.subtract)
                cw2 = smt("cw2")
                V.tensor_tensor(cw2[:], cw[:], cw[:], ALU.mult)
                ch2 = smt("ch2")
                V.tensor_tensor(ch2[:], ch[:], ch[:], ALU.mult)
                c2 = smt("c2")
                V.tensor_tensor(c2[:], cw2[:], ch2[:], ALU.add)
                V.tensor_scalar(c2[:], c2[:], 1e-7, None, ALU.add)
                # rho2
                dx = smt("dx")
                V.tensor_tensor(e1[:], tx1, tx2, ALU.add)
                V.tensor_tensor(dx[:], psx[:], e1[:], ALU.subtract)
                V.tensor_tensor(dx[:], dx[:], dx[:], ALU.mult)
                dy = smt("dy")
                V.tensor_tensor(e1[:], ty1, ty2, ALU.add)
                V.tensor_tensor(dy[:], psy[:], e1[:], ALU.subtract)
                V.tensor_tensor(dy[:], dy[:], dy[:], ALU.mult)
                rho2 = smt("rho2")
                V.tensor_tensor(rho2[:], dx[:], dy[:], ALU.add)
                V.tensor_scalar(rho2[:], rho2[:], 0.25, None, ALU.mult)
                # dw2, dh2
                dw = smt("dw")
                V.tensor_tensor(dw[:], pw[:], tw[:], ALU.subtract)
                V.tensor_tensor(dw[:], dw[:], dw[:], ALU.mult)
                dh = smt("dh")
                V.tensor_tensor(dh[:], ph[:], th[:], ALU.subtract)
                V.tensor_tensor(dh[:], dh[:], dh[:], ALU.mult)
                # assemble: e = 1 - iou + rho2/c2 + dw2/(cw2+eps) + dh2/(ch2+eps)
                acc = smt("acc")
                V.tensor_scalar(acc[:], iouv[:], -1.0, 1.0, ALU.mult, ALU.add)
                V.reciprocal(r_[:], c2[:])
                V.tensor_tensor(rho2[:], rho2[:], r_[:], ALU.mult)
                V.tensor_tensor(acc[:], acc[:], rho2[:], ALU.add)
                V.tensor_scalar(cw2[:], cw2[:], 1e-7, None, ALU.add)
                V.reciprocal(r_[:], cw2[:])
                V.tensor_tensor(dw[:], dw[:], r_[:], ALU.mult)
                V.tensor_tensor(acc[:], acc[:], dw[:], ALU.add)
                V.tensor_scalar(ch2[:], ch2[:], 1e-7, None, ALU.add)
                V.reciprocal(r_[:], ch2[:])
                V.tensor_tensor(dh[:], dh[:], r_[:], ALU.mult)
                V.tensor_tensor(acc[:], acc[:], dh[:], ALU.add)
                # masked sums
                V.tensor_tensor(acc[:], acc[:], fg[:], ALU.mult)
                V.tensor_reduce(res[:, ecol:ecol + 1], acc[:], AXL.X, ALU.add)
                V.tensor_reduce(res[:, ccol:ccol + 1], fg[:], AXL.X, ALU.add)

            tb = img_out["tgtb"]
            eiou(tb[:, :, 0], tb[:, :, 2], tb[:, :, 1], tb[:, :, 3],
                 img_out["fg0"], 8 + i, 12 + i)
            eiou(tdp[1][:], tdp[3][:], tdp[2][:], tdp[4][:],
                 img_out["fg1"], 16 + i, 20 + i)

        # ============ final reduction & scalar math ============
        psres = psum_s.tile([1, 24], F32, tag="sps", name="sps")
        PE.matmul(psres[:], ones1[:], res[:], start=True, stop=True)
        resr = sm.tile([1, 24], F32, tag="resr", name="resr")
        S.activation(resr[:], psres[:], ACTF.Copy)
        DMA.dma_start(d_res.ap(), resr[:])
        # conf = F1*(0.25/8400) + F2*(0.75/8400)
        conf = sm.tile([1, 4], F32, tag="conf", name="conf")
        V.tensor_scalar(conf[:], resr[0:1, 0:4], 0.25 / 8400.0, None, ALU.mult)
        tmp4 = sm.tile([1, 4], F32, tag="tmp4", name="tmp4")
        V.tensor_scalar(tmp4[:], resr[0:1, 4:8], 0.75 / 8400.0, None, ALU.mult)
        V.tensor_tensor(conf[:], conf[:], tmp4[:], ALU.add)
        # bbox = 0.25*E1/max(C1,1) + 0.75*E2/max(C2,1)
        bbox = sm.tile([1, 4], F32, tag="bbox", name="bbox")
        c1m = sm.tile([1, 4], F32, tag="c1m", name="c1m")
        V.tensor_scalar(c1m[:], resr[0:1, 12:16], 1.0, None, ALU.max)
        rc4 = sm.tile([1, 4], F32, tag="rc4", name="rc4")
        V.reciprocal(rc4[:], c1m[:])
        V.tensor_tensor(bbox[:], resr[0:1, 8:12], rc4[:], ALU.mult)
        V.tensor_scalar(bbox[:], bbox[:], 0.25, None, ALU.mult)
        V.tensor_scalar(c1m[:], resr[0:1, 20:24], 1.0, None, ALU.max)
        V.reciprocal(rc4[:], c1m[:])
        V.tensor_tensor(tmp4[:], resr[0:1, 16:20], rc4[:], ALU.mult)
        V.tensor_scalar(tmp4[:], tmp4[:], 0.75, None, ALU.mult)
        V.tensor_tensor(bbox[:], bbox[:], tmp4[:], ALU.add)
        loss4 = sm.tile([1, 4], F32, tag="loss4", name="loss4")
        V.scalar_tensor_tensor(loss4[:], bbox[:], 5.0, conf[:], ALU.mult, ALU.add)
        DMA.dma_start(d_out.ap(), loss4[:])

    nc.compile()
    return nc


# ==================== host side ====================

def _prep_core(inputs, core):
    f32 = np.float32
    sl = slice(core * 4, core * 4 + 4)

    def tile_plane(v):  # [8448] -> [128, 66]
        return np.ascontiguousarray(v.reshape(T, 128).T)

    sd = np.asarray(inputs["student_decoded_bboxes"][sl], f32)
    td = np.asarray(inputs["teacher_decoded_bboxes"][sl], f32)
    sp = np.asarray(inputs["student_predictions"][sl, :, 0], f32)
    tp = np.asarray(inputs["teacher_predictions"][sl, :, 0], f32)
    tg = np.asarray(inputs["targets"][sl], f32)

    def planes5(dec):
        out = np.zeros((NIMG, 5, 128, T), f32)
        for i in range(NIMG):
            for j in range(5):
                v = np.zeros(AP_, f32)
                v[:A] = dec[i, :, j]
                if j == 0:
                    v[A:] = 1.0
                out[i, j] = tile_plane(v)
        return out

    def plane1(x):
        out = np.zeros((NIMG, 128, T), f32)
        for i in range(NIMG):
            v = np.full(AP_, -40.0, f32)
            v[:A] = x[i]
            out[i] = tile_plane(v)
        return out

    return {
        "sd": planes5(sd), "td": planes5(td),
        "sp": plane1(sp), "tp": plane1(tp),
        "tg": np.ascontiguousarray(tg),
    }


def _const_inputs(priors):
    f32 = np.float32
    pri = np.asarray(priors, f32)
    pp = np.zeros((4, 128, T), f32)
    for j in range(4):
        v = np.full(AP_, [-1e6, -1e6, 1.0, 1.0][j], f32)
        v[:A] = pri[:, j]
        pp[j] = np.ascontiguousarray(v.reshape(T, 128).T)
    eye = np.eye(128, dtype=f32)
    iotam = np.broadcast_to(np.arange(M, dtype=f32), (128, M)).copy()
    iotap = np.zeros((128, 1), f32)
    for p in range(128):
        if p < 48:
            iotap[p] = p
        elif 64 <= p < 112:
            iotap[p] = p - 64
        else:
            iotap[p] = 1000.0
    iota8 = np.broadcast_to(np.arange(8, dtype=f32), (128, 8)).copy()
    return {"pri": pp, "eye": eye, "iotam": iotam, "iotap": iotap,
            "iota8": iota8}


def build_in_maps(inputs):
    consts = _const_inputs(inputs["student_priors"])
    in_maps = []
    for core in range(8):
        m = _prep_core(inputs, core)
        m.update(consts)
        in_maps.append(m)
    return in_maps


def kernel(**inputs):
    from concourse.bass_utils import run_bass_kernel_spmd
    if "nc" not in _CACHED:
        _CACHED["nc"] = build_nc()
    nc = _CACHED["nc"]
    in_maps = build_in_maps(inputs)
    res = run_bass_kernel_spmd(nc, in_maps, core_ids=list(range(8)))
    losses = np.concatenate([r["out_losses"].ravel() for r in res.results])
    return np.float32(np.mean(losses))


if __name__ == "__main__":
    import reference
    inputs = {k: np.asarray(v) for k, v in reference.setup_inputs().items()}
    out = kernel(**inputs)
    print("kernel loss:", out)

